# revision 41
# baseline (speedup 1.0000x reference)
"""Distributed causal single-head attention kernel for 8 TRN2 NeuronCores.

Problem (hardcoded): x [4, 2048, 1024], Wq/Wk/Wv [1024, 1024] (torch Linear
layout, y = x @ W.T), causal softmax attention, out [4, 2048, 1024] f32.

Sharding: 8 cores = 4 batches x 2 cores. Each core owns 1024 query rows of
one batch (folded pairing of 512-row blocks {0,3}/{1,2} balances causal
work). K and V are NEVER materialized: the weight projections are
reassociated onto the query side, which eliminates the pair's key-side
duplication entirely:
    R   = Wk^T @ Q^T          (Wk-projected queries, query-side)
    S^T = x_kd @ R            (scores contract x directly)
    Z_g = x_kd^T @ P_g        (key-contraction first)
    O^T = Wv @ Z_g            (Wv applied on the query side)
Other structure:
  - host-side key gather so each core's q rows sit at fixed positions
    (0:512 and 1536:2048) of its gathered x -> one uniform SPMD program
  - scores computed transposed so softmax probabilities are directly the
    moving operand of the Z matmul (no on-device transposes)
  - no max-subtraction softmax (scores are ~N(0,1); max |s| ~ 8, exp safe)
  - exact-causal tiling (moving-cycle optimal at 128-key-block granularity):
      12 full 512-wide score tiles: GF (key blocks 4..7 x rflex, where
        rflex = R_A or R_B selected on device with 0/1 weights from data)
        and GBF (key blocks {0..3, 8..11} x R_B);
      20 diagonal 128-wide sub-tiles (group A x R_A, group B x R_B), the 8
        true-diagonal ones masked with ONE shared 128x128 lower-tri mask.
    GF's Z/rowsum accumulate separately and are combined into the A or B
    paths with the same 0/1 weights, so O needs only two groups.
  - rowsums: DVE pre-accumulates P tiles per group in f32 (off the PE
    critical path), then a single ones-matmul per group reduces partitions;
    normalization (divide by rowsum) on host.
  - all SBUF/PSUM tiles live in flat persistent pools (per-tag rotating
    bufs): no pool-scope close barriers, so in a repeated-body graph the
    next body's input DMAs prefetch during the current body's tail. Output
    stores issue on the Activation HWDGE queue to keep the SP queue free
    for input loads.
"""

import sys
import numpy as np

for _p in ("/opt/trn_rl_repo",):
    if _p not in sys.path:
        sys.path.insert(0, _p)

import ml_dtypes

B, S, D = 4, 2048, 1024
QB = 512          # q-tile width (2 per core)
KB = 128          # key block
NKB = S // KB     # 16 key blocks
ND = D // 128     # 8 d-slices
QPOS = (0, 1536)  # positions of the two q blocks inside the gathered key axis
GF = tuple(range(4, 8))               # flex-group key blocks (full tiles)
GBF = (0, 1, 2, 3, 8, 9, 10, 11)      # B-group full-tile key blocks
# diagonal sub-tiles at 128x128: (group, qj, i); key block = (0|12)+i,
# q sub-col qj of slot A|B; i==qj gets the shared lower-tri mask
SUBS = tuple((g, qj, i) for g in (0, 1) for qj in range(4)
             for i in range(qj + 1))
NFULL = 4 + len(GBF)                  # 12 full tiles (GF + GBF)
SUBBASE = NFULL * QB                  # p_bf column offset of sub-tiles
N_CORES = 8


def _sidx(g, qj, i):
    return g * 10 + qj * (qj + 1) // 2 + i

_SCALE = 1.0 / float(np.sqrt(np.float32(D)))


def _core_layout(core):
    """(batch, [qblock row-block indices], key gather order, wa)."""
    b, t = core // 2, core % 2
    if t == 0:
        qbs = [0, 3]
        order = [0, 1, 2, 3]
        wa = 0.0
    else:
        qbs = [1, 2]
        order = [1, 0, 0, 2]
        wa = 1.0
    gather = np.concatenate([np.arange(o * QB, (o + 1) * QB) for o in order])
    return b, qbs, gather, wa


def build_nc(body_reps=1):
    """Build the SPMD Bass graph (same program for all 8 cores)."""
    import concourse.tile as tile
    import concourse.mybir as mybir
    from concourse import bacc
    from contextlib import ExitStack

    fp32 = mybir.dt.float32
    bf16 = mybir.dt.bfloat16

    nc = bacc.Bacc("TRN2", target_bir_lowering=False, debug=False)

    xT = nc.dram_tensor("xT", [D, S], bf16, kind="ExternalInput").ap()
    xkd = nc.dram_tensor("xkd", [S, D], bf16, kind="ExternalInput").ap()
    gT = nc.dram_tensor("gT", [D, D], bf16, kind="ExternalInput").ap()
    wvT = nc.dram_tensor("wvT", [D, D], bf16, kind="ExternalInput").ap()
    trimask = nc.dram_tensor("trimask", [128, 128], bf16,
                             kind="ExternalInput").ap()
    wsel = nc.dram_tensor("wsel", [KB, 2], fp32, kind="ExternalInput").ap()
    wlsel = nc.dram_tensor("wlsel", [2, QB], fp32, kind="ExternalInput").ap()
    outT = nc.dram_tensor("outT", [D, 2 * QB], fp32, kind="ExternalOutput").ap()
    lsum = nc.dram_tensor("lsum", [2, QB], fp32, kind="ExternalOutput").ap()

    xT_r = xT.rearrange("(a p) s -> a p s", p=128)       # [8, 128, 2048]
    xT_p = xT.rearrange("(a p) s -> p a s", p=128)       # [128, 8, 2048]
    xkd_p = xkd.rearrange("(kb p) d -> p kb d", p=128)   # [128, 16, 1024]
    g_r = gT.rearrange("(a p) d -> a p d", p=128)
    wv_p = wvT.rearrange("(a p) d -> p a d", p=128)
    outT_r = outT.rearrange("(a p) q -> a p q", p=128)   # [8, 128, 1024]

    QW = 2 * QB  # 1024 q rows per core

    # full-tile schedule: (key block, r source 1=R_B 2=rflex, p full-col)
    FSCHED = ([(kb, 1, 4 + i) for i, kb in enumerate(GBF)] +
              [(kb, 2, i) for i, kb in enumerate(GF)])
    NPC = SUBBASE + len(SUBS) * 128      # p_bf columns: 6144 + 2560

    with tile.TileContext(nc) as tc:
        with ExitStack() as root:
            const = root.enter_context(tc.tile_pool(name="const", bufs=1))
            ones_bf = const.tile([128, 1], bf16)
            nc.vector.memset(ones_bf[:], 1.0)
            ws = const.tile([KB, 2], fp32)
            wls_a = const.tile([1, QB], fp32, tag="wlsa")
            wls_b = const.tile([1, QB], fp32, tag="wlsb")

            # Everything persistent: no per-rep pool scopes, so the only
            # cross-rep ordering is fine-grained WAR on the tiles themselves
            # (pool-scope close barriers chained rep N+1's loads behind rep
            # N's out-store drain).
            persist = root.enter_context(tc.tile_pool(name="persist", bufs=1))
            xt_bf = persist.tile([128, ND * S], bf16, tag="xt")     # 32KB/part
            xkd_bf = persist.tile([128, NKB * D], bf16, tag="xkd")  # 32KB/part
            w_bf = persist.tile([128, ND * D], bf16, tag="w")       # 16KB/part
            rt = persist.tile([128, ND * QW], bf16, tag="rt")       # 16KB/part
            rfx = persist.tile([128, ND * QB], bf16, tag="rfx")     # 8KB/part
            mk = persist.tile([128, 128], bf16, tag="mk")           # 0.25KB/part
            wv_bf = persist.tile([128, ND * D], bf16, tag="wv")     # 16KB/part
            za = persist.tile([128, ND * QB], bf16, tag="za")       # 8KB/part
            zb = persist.tile([128, ND * QB], bf16, tag="zb")       # 8KB/part
            zf = persist.tile([128, ND * QB], bf16, tag="zf")       # 8KB/part
            p_bf = persist.tile([128, NPC], bf16, tag="p")          # 17KB/part
            # f32 rowsum pre-accumulators (DVE), one per softmax group
            pacc_f = persist.tile([128, QB], fp32, tag="paf")       # 2KB/part
            pacc_a = persist.tile([128, QB], fp32, tag="paa")       # 2KB/part
            pacc_b = persist.tile([128, QB], fp32, tag="pab")       # 2KB/part

            # rotating scratch (per-tag bufs)
            scr = root.enter_context(tc.tile_pool(name="scr", bufs=2))
            # one PSUM pool: shared rotating tag (5 banks) + 3 rowsum banks
            psum = root.enter_context(
                tc.tile_pool(name="psum", bufs=5, space="PSUM"))

            for rep in range(body_reps):
                # ---------- phase A: load, Q-proj, R-proj ----------
                # DMA order: tiny first chunks, then wq interleaved with
                # the xt q-columns that Q-proj consumes first. (All loads on
                # the SP queue: the opening is HWDGE-descriptor-generation
                # bound, shared across queues, so striping SP/Act does not
                # raise delivery rate — measured in TimelineSim.)
                xt_v = xt_bf[:].rearrange("p (a s) -> p a s", a=ND)
                nc.sync.dma_start(w_bf[:, 0:128], g_r[0][:, 0:128])
                nc.sync.dma_start(
                    xt_bf[:, QPOS[0]: QPOS[0] + QB],
                    xT_r[0][:, QPOS[0]:QPOS[0] + QB])
                nc.sync.dma_start(w_bf[:, 128:D], g_r[0][:, 128:D])
                for a in range(1, ND):
                    nc.sync.dma_start(w_bf[:, a * D:(a + 1) * D], g_r[a])
                    nc.sync.dma_start(
                        xt_bf[:, a * S + QPOS[0]: a * S + QPOS[0] + QB],
                        xT_r[a][:, QPOS[0]:QPOS[0] + QB])
                for a in range(ND):
                    nc.sync.dma_start(
                        xt_bf[:, a * S + QPOS[1]: a * S + QPOS[1] + QB],
                        xT_r[a][:, QPOS[1]:QPOS[1] + QB])
                nc.sync.dma_start(
                    xt_v[:, :, QB:QPOS[1]], xT_p[:, :, QB:QPOS[1]])
                if rep == 0:
                    nc.sync.dma_start(ws[:], wsel[:])
                    nc.sync.dma_start(wls_a[:], wlsel[0:1, :])
                    nc.sync.dma_start(wls_b[:], wlsel[1:2, :])

                # xkd, wv, mask batched (consumed much later)
                nc.sync.dma_start(
                    xkd_bf[:].rearrange("p (kb d) -> p kb d", kb=NKB),
                    xkd_p)
                nc.sync.dma_start(
                    wv_bf[:].rearrange("p (a d) -> p a d", a=ND), wv_p)
                if rep == 0:
                    nc.sync.dma_start(mk[:], trimask)

                # R [din, q] = G^T @ x_q^T with G = Wk^T Wq folded on host.
                # a-outer over 4-bank din groups: the first matmul needs only
                # the first w/xt chunk, so PE starts ~1 us into the load
                # instead of after the full 2 MB of w.
                for qc in range(2):
                    qp = QPOS[qc]
                    for dg in range(0, ND, 4):
                        pts = [psum.tile([128, QB], fp32, tag="ps",
                                         name=f"pr{qc}_{dg}_{di}")
                               for di in range(4)]
                        for a in range(ND):
                            for di in range(4):
                                nc.tensor.matmul(
                                    pts[di][:],
                                    w_bf[:, a * D + (dg + di) * 128:
                                         a * D + (dg + di) * 128 + 128],
                                    xt_bf[:, a * S + qp: a * S + qp + QB],
                                    start=(a == 0), stop=(a == ND - 1))
                        for di in range(4):
                            din = dg + di
                            nc.vector.tensor_copy(
                                rt[:, din * QW + qc * QB:
                                   din * QW + qc * QB + QB],
                                pts[di][:])

                # rflex = wa*R_A + wb*R_B (wa, wb in {0,1} from data).
                # All on DVE: Act's in-order stream must stay clear for the
                # score exps (PSUM backpressure).
                for a in range(ND):
                    ra = rt[:, a * QW: a * QW + QB]
                    rb = rt[:, a * QW + QB: a * QW + 2 * QB]
                    t1 = scr.tile([128, QB], bf16, tag="t1")
                    nc.vector.tensor_scalar_mul(t1[:], ra, ws[:, 0:1])
                    t2 = scr.tile([128, QB], bf16, tag="t2")
                    nc.vector.tensor_scalar_mul(t2[:], rb, ws[:, 1:2])
                    nc.vector.tensor_add(
                        rfx[:, a * QB:(a + 1) * QB], t1[:], t2[:])

                # ---------- phase B1: scores + exp + rowsums ----------
                def rsrc_ap(qs, a):
                    # qs: 0 = R_A, 1 = R_B, 2 = rflex
                    if qs == 0:
                        return rt[:, a * QW: a * QW + QB]
                    if qs == 1:
                        return rt[:, a * QW + QB: a * QW + 2 * QB]
                    return rfx[:, a * QB:(a + 1) * QB]

                def pfull_ap(fcol):
                    return p_bf[:, fcol * QB:(fcol + 1) * QB]

                def psub_ap(g, qj, i):
                    c = SUBBASE + _sidx(g, qj, i) * 128
                    return p_bf[:, c:c + 128]

                # diagonal sub-tiles at 128 granularity (A group first —
                # R_A lands first), packed 4 per PSUM bank
                def sub_scores(g):
                    grp = [(qj, i) for qj in range(4) for i in range(qj + 1)]
                    for base in range(0, len(grp), 4):
                        pack = grp[base:base + 4]
                        pst = psum.tile([128, QB], fp32, tag="ps")
                        for r, (qj, i) in enumerate(pack):
                            kb = (0 if g == 0 else 12) + i
                            for a in range(ND):
                                rs = rsrc_ap(g, a)
                                nc.tensor.matmul(
                                    pst[:, r * 128:(r + 1) * 128],
                                    xt_bf[:, a * S + kb * 128: a * S + kb * 128 + 128],
                                    rs[:, qj * 128:(qj + 1) * 128],
                                    start=(a == 0), stop=(a == ND - 1),
                                    skip_group_check=True)
                        for r, (qj, i) in enumerate(pack):
                            pc = psub_ap(g, qj, i)
                            nc.scalar.activation(
                                pc, pst[:, r * 128:(r + 1) * 128],
                                mybir.ActivationFunctionType.Exp,
                                scale=_SCALE)
                            if i == qj:
                                nc.vector.tensor_mul(pc, pc, mk[:])

                sub_scores(0)
                for kb, qs, fcol in FSCHED:
                    pst = psum.tile([128, QB], fp32, tag="ps")
                    for a in range(ND):
                        nc.tensor.matmul(
                            pst[:],
                            xt_bf[:, a * S + kb * 128: a * S + kb * 128 + 128],
                            rsrc_ap(qs, a),
                            start=(a == 0), stop=(a == ND - 1))
                    nc.scalar.activation(
                        pfull_ap(fcol), pst[:],
                        mybir.ActivationFunctionType.Exp,
                        scale=_SCALE)
                    if kb == GBF[-1] and qs == 1:
                        sub_scores(1)

                # rowsums: DVE pre-accumulates the P tiles per group (f32,
                # off the PE critical path), then one ones-matmul per group
                # reduces partitions — 3x512 PE cycles instead of 8704.
                def accsubs(pacc, g):
                    for qj in range(4):
                        seg = pacc[:, qj * 128:(qj + 1) * 128]
                        for i in range(qj + 1):
                            if g == 0 and i == 0:
                                nc.vector.tensor_copy(seg, psub_ap(g, qj, i))
                            else:
                                nc.vector.tensor_add(
                                    seg, seg, psub_ap(g, qj, i))

                def lreduce(pacc, tag, btag):
                    pb = scr.tile([128, QB], bf16, tag=btag, bufs=1,
                                  name=f"pb_{btag}")
                    nc.vector.tensor_copy(pb[:], pacc[:])
                    plt = psum.tile([1, QB], fp32, tag=tag, bufs=1,
                                    name=f"l_{tag}")
                    nc.tensor.matmul(plt[:], ones_bf[:], pb[:],
                                     start=True, stop=True)
                    return plt

                # ------- phase B2: Z = x^T P (+combine), O = Wv Z -------
                def xk_ap(kb, dsl):
                    return xkd_bf[:, kb * D + dsl * 128: kb * D + dsl * 128 + 128]

                def zsubs(pz, g, dsl, start_open):
                    for qj in range(4):
                        for i in range(qj + 1):
                            kb = (0 if g == 0 else 12) + i
                            nc.tensor.matmul(
                                pz[:, qj * 128:(qj + 1) * 128],
                                xk_ap(kb, dsl), psub_ap(g, qj, i),
                                start=(i == 0 and not start_open),
                                stop=(i == qj),
                                skip_group_check=True)

                def zcombine(zt, pz, dsl, wcol):
                    tf = scr.tile([128, QB], bf16, tag="tf")
                    nc.vector.tensor_scalar_mul(
                        tf[:], zf[:, dsl * QB:(dsl + 1) * QB],
                        ws[:, wcol:wcol + 1])
                    nc.vector.tensor_add(
                        zt[:, dsl * QB:(dsl + 1) * QB], pz[:], tf[:])

                # Z_F first (plain copy; zf feeds both combines)
                for dsl in range(ND):
                    pz = psum.tile([128, QB], fp32, tag="ps")
                    for j, kb in enumerate(GF):
                        nc.tensor.matmul(
                            pz[:], xk_ap(kb, dsl), pfull_ap(j),
                            start=(j == 0), stop=(j == 3))
                    nc.vector.tensor_copy(
                        zf[:, dsl * QB:(dsl + 1) * QB], pz[:])

                # Z_A: sub-tiles only
                for dsl in range(ND):
                    pz = psum.tile([128, QB], fp32, tag="ps")
                    zsubs(pz, 0, dsl, start_open=False)
                    zcombine(za, pz, dsl, 0)

                # rowsum pre-accumulation on DVE — emitted here so it drains
                # during Z_B's long PE chains instead of stalling Z_A
                accsubs(pacc_a, 0)
                nc.vector.tensor_add(
                    pacc_b[:], pfull_ap(4), pfull_ap(5))
                for j in range(2, len(GBF)):
                    nc.vector.tensor_add(
                        pacc_b[:], pacc_b[:], pfull_ap(4 + j))
                accsubs(pacc_b, 1)
                nc.vector.tensor_add(pacc_f[:], pfull_ap(0), pfull_ap(1))
                for j in range(2, 4):
                    nc.vector.tensor_add(
                        pacc_f[:], pacc_f[:], pfull_ap(j))

                # Z_B: full part then sub-tiles
                for dsl in range(ND):
                    pz = psum.tile([128, QB], fp32, tag="ps")
                    for j, kb in enumerate(GBF):
                        nc.tensor.matmul(
                            pz[:], xk_ap(kb, dsl), pfull_ap(4 + j),
                            start=(j == 0), stop=False,
                            skip_group_check=True)
                    zsubs(pz, 1, dsl, start_open=True)
                    zcombine(zb, pz, dsl, 1)

                # rowsum reduce + lsum store (PE cost: 3x512 cycles)
                l_f = lreduce(pacc_f, "lf", "lfb")
                l_a = lreduce(pacc_a, "la", "lab")
                l_b = lreduce(pacc_b, "lb", "lbb")
                for s, l_s, wl in ((0, l_a, wls_a), (1, l_b, wls_b)):
                    tl = scr.tile([1, QB], fp32, tag="tl")
                    nc.vector.tensor_mul(tl[:], l_f[:], wl[:])
                    lo = scr.tile([1, QB], fp32, tag="lo")
                    nc.vector.tensor_add(lo[:], l_s[:], tl[:])
                    # stores go on the Activation HWDGE queue so the SP
                    # queue's next-rep input loads aren't head-blocked
                    nc.scalar.dma_start(lsum[s:s + 1, :], lo[:])

                # O^T [dv, q] = Wv @ Z (unnormalized; host divides).
                # dv-outer so both q-slots of a dv pair into ONE [128,1024]
                # tile and one store: 8 stores instead of 16 halves the
                # HWDGE store-stream processing that can pace the O phase
                # at real (faster-than-model) PE clocks.
                for dv in range(ND):
                    ot = scr.tile([128, 2 * QB], fp32, tag="ot", bufs=2)
                    for s, zt in ((0, za), (1, zb)):
                        po = psum.tile([128, QB], fp32, tag="ps")
                        for a in range(ND):
                            nc.tensor.matmul(
                                po[:],
                                wv_bf[:, a * D + dv * 128: a * D + dv * 128 + 128],
                                zt[:, a * QB:(a + 1) * QB],
                                start=(a == 0), stop=(a == ND - 1))
                        nc.vector.tensor_copy(
                            ot[:, s * QB:(s + 1) * QB], po[:])
                    nc.scalar.dma_start(outT_r[dv], ot[:])

    nc.compile()
    return nc


_NC_CACHE = {}


def _get_nc(body_reps=1):
    if body_reps not in _NC_CACHE:
        _NC_CACHE[body_reps] = build_nc(body_reps)
    return _NC_CACHE[body_reps]


def make_in_maps(x, Wq, Wk, Wv):
    """Host-side sharding: per-core input dict."""
    x = np.asarray(x, dtype=np.float32)
    gTn = np.ascontiguousarray(
        np.asarray(Wq, np.float64).T @ np.asarray(Wk, np.float64)
    ).astype(ml_dtypes.bfloat16)
    wvT = np.ascontiguousarray(np.asarray(Wv, np.float32).T).astype(ml_dtypes.bfloat16)

    # shared lower-tri (keep k <= q) mask for every diagonal 128x128 block
    tri = (np.arange(128)[:, None] <= np.arange(128)[None, :]).astype(
        ml_dtypes.bfloat16)

    in_maps = []
    for core in range(N_CORES):
        b, qbs, gather, wa = _core_layout(core)
        xg = x[b][gather]                                    # [S, D] gathered
        xkd = np.ascontiguousarray(xg).astype(ml_dtypes.bfloat16)
        xTp = np.ascontiguousarray(xg.T).astype(ml_dtypes.bfloat16)
        wsel = np.zeros((KB, 2), np.float32)
        wsel[:, 0] = wa
        wsel[:, 1] = 1.0 - wa
        wlsel = np.zeros((2, QB), np.float32)
        wlsel[0, :] = wa
        wlsel[1, :] = 1.0 - wa
        in_maps.append({
            "xT": xTp,
            "xkd": xkd,
            "gT": gTn,
            "wvT": wvT,
            "trimask": tri,
            "wsel": wsel,
            "wlsel": wlsel,
        })
    return in_maps


def assemble_output(results):
    out = np.zeros((B, S, D), np.float32)
    for core in range(N_CORES):
        b, qbs, _, _ = _core_layout(core)
        outT = results[core]["outT"]      # [D, 1024] unnormalized
        l = results[core]["lsum"]         # [2, QB]
        for slot in range(2):
            rows = np.arange(qbs[slot] * QB, (qbs[slot] + 1) * QB)
            o = outT[:, slot * QB:(slot + 1) * QB].T   # [QB, D]
            out[b, rows, :] = o / l[slot][:, None]
    return out


def kernel(x, Wq, Wk, Wv):
    from concourse.bass_utils import run_bass_kernel_spmd
    nc = _get_nc()
    in_maps = make_in_maps(x, Wq, Wk, Wv)
    res = run_bass_kernel_spmd(nc, in_maps, core_ids=list(range(N_CORES)))
    return assemble_output(res.results)



# revision 43
# speedup vs baseline: 1.4438x; 1.4438x over previous
"""Distributed causal single-head attention kernel for 8 TRN2 NeuronCores.

Problem (hardcoded): x [4, 2048, 1024], Wq/Wk/Wv [1024, 1024] (torch Linear
layout, y = x @ W.T), causal softmax attention, out [4, 2048, 1024] f32.

Sharding: 8 cores = 4 batches x 2 cores. Each core owns 1024 query rows of
one batch (folded pairing of 512-row blocks {0,3}/{1,2} balances causal
work). K and V are NEVER materialized: the weight projections are
reassociated onto the query side, which eliminates the pair's key-side
duplication entirely:
    R   = Wk^T @ Q^T          (Wk-projected queries, query-side)
    S^T = x_kd @ R            (scores contract x directly)
    Z_g = x_kd^T @ P_g        (key-contraction first)
    O^T = Wv @ Z_g            (Wv applied on the query side)
Other structure:
  - host-side key gather so each core's q rows sit at fixed positions
    (0:512 and 1536:2048) of its gathered x -> one uniform SPMD program
  - scores computed transposed so softmax probabilities are directly the
    moving operand of the Z matmul (no on-device transposes)
  - no max-subtraction softmax (scores are ~N(0,1); max |s| ~ 8, exp safe)
  - exact-causal tiling (moving-cycle optimal at 128-key-block granularity):
      12 full 512-wide score tiles: GF (key blocks 4..7 x rflex, where
        rflex = R_A or R_B selected on device with 0/1 weights from data)
        and GBF (key blocks {0..3, 8..11} x R_B);
      20 diagonal 128-wide sub-tiles (group A x R_A, group B x R_B), the 8
        true-diagonal ones masked with ONE shared 128x128 lower-tri mask.
    GF's Z/rowsum accumulate separately and are combined into the A or B
    paths with the same 0/1 weights, so O needs only two groups.
  - rowsums: DVE pre-accumulates P tiles per group in f32 (off the PE
    critical path), then a single ones-matmul per group reduces partitions;
    normalization (divide by rowsum) on host.
  - all SBUF/PSUM tiles live in flat persistent pools (per-tag rotating
    bufs): no pool-scope close barriers, so in a repeated-body graph the
    next body's input DMAs prefetch during the current body's tail. Output
    stores issue on the Activation HWDGE queue to keep the SP queue free
    for input loads.
"""

import sys
import numpy as np

for _p in ("/opt/trn_rl_repo",):
    if _p not in sys.path:
        sys.path.insert(0, _p)

import ml_dtypes

B, S, D = 4, 2048, 1024
QB = 512          # q-tile width (2 per core)
KB = 128          # key block
NKB = S // KB     # 16 key blocks
ND = D // 128     # 8 d-slices
QPOS = (0, 1536)  # positions of the two q blocks inside the gathered key axis
GF = tuple(range(4, 8))               # flex-group key blocks (full tiles)
GBF = (0, 1, 2, 3, 8, 9, 10, 11)      # B-group full-tile key blocks
# diagonal sub-tiles at 128x128: (group, qj, i); key block = (0|12)+i,
# q sub-col qj of slot A|B; i==qj gets the shared lower-tri mask
SUBS = tuple((g, qj, i) for g in (0, 1) for qj in range(4)
             for i in range(qj + 1))
NFULL = 4 + len(GBF)                  # 12 full tiles (GF + GBF)
SUBBASE = NFULL * QB                  # p_bf column offset of sub-tiles
N_CORES = 8


def _sidx(g, qj, i):
    return g * 10 + qj * (qj + 1) // 2 + i

_SCALE = 1.0 / float(np.sqrt(np.float32(D)))


def _core_layout(core):
    """(batch, [qblock row-block indices], key gather order, wa)."""
    b, t = core // 2, core % 2
    if t == 0:
        qbs = [0, 3]
        order = [0, 1, 2, 3]
        wa = 0.0
    else:
        qbs = [1, 2]
        order = [1, 0, 0, 2]
        wa = 1.0
    gather = np.concatenate([np.arange(o * QB, (o + 1) * QB) for o in order])
    return b, qbs, gather, wa


def build_nc(body_reps=1):
    """Build the SPMD Bass graph (same program for all 8 cores)."""
    import concourse.tile as tile
    import concourse.mybir as mybir
    from concourse import bacc
    from contextlib import ExitStack

    fp32 = mybir.dt.float32
    bf16 = mybir.dt.bfloat16

    nc = bacc.Bacc("TRN2", target_bir_lowering=False, debug=False)

    xT = nc.dram_tensor("xT", [D, S], bf16, kind="ExternalInput").ap()
    xkd = nc.dram_tensor("xkd", [S, D], bf16, kind="ExternalInput").ap()
    gT = nc.dram_tensor("gT", [D, D], bf16, kind="ExternalInput").ap()
    wvT = nc.dram_tensor("wvT", [D, D], bf16, kind="ExternalInput").ap()
    trimask = nc.dram_tensor("trimask", [128, 128], bf16,
                             kind="ExternalInput").ap()
    wsel = nc.dram_tensor("wsel", [KB, 2], fp32, kind="ExternalInput").ap()
    wlsel = nc.dram_tensor("wlsel", [2, QB], fp32, kind="ExternalInput").ap()
    outT = nc.dram_tensor("outT", [D, 2 * QB], fp32, kind="ExternalOutput").ap()
    lsum = nc.dram_tensor("lsum", [2, QB], fp32, kind="ExternalOutput").ap()

    xT_r = xT.rearrange("(a p) s -> a p s", p=128)       # [8, 128, 2048]
    xT_p = xT.rearrange("(a p) s -> p a s", p=128)       # [128, 8, 2048]
    xkd_p = xkd.rearrange("(kb p) d -> p kb d", p=128)   # [128, 16, 1024]
    g_r = gT.rearrange("(a p) d -> a p d", p=128)
    wv_p = wvT.rearrange("(a p) d -> p a d", p=128)
    outT_r = outT.rearrange("(a p) q -> a p q", p=128)   # [8, 128, 1024]

    QW = 2 * QB  # 1024 q rows per core

    # full-tile schedule: (key block, r source 1=R_B 2=rflex, p full-col)
    FSCHED = ([(kb, 1, 4 + i) for i, kb in enumerate(GBF)] +
              [(kb, 2, i) for i, kb in enumerate(GF)])
    NPC = SUBBASE + len(SUBS) * 128      # p_bf columns: 6144 + 2560

    with tile.TileContext(nc) as tc:
        with ExitStack() as root:
            const = root.enter_context(tc.tile_pool(name="const", bufs=1))
            ones_bf = const.tile([128, 1], bf16)
            nc.vector.memset(ones_bf[:], 1.0)
            ws = const.tile([KB, 2], fp32)
            wls_a = const.tile([1, QB], fp32, tag="wlsa")
            wls_b = const.tile([1, QB], fp32, tag="wlsb")

            # Everything persistent: no per-rep pool scopes, so the only
            # cross-rep ordering is fine-grained WAR on the tiles themselves
            # (pool-scope close barriers chained rep N+1's loads behind rep
            # N's out-store drain).
            persist = root.enter_context(tc.tile_pool(name="persist", bufs=1))
            xt_bf = persist.tile([128, ND * S], bf16, tag="xt")     # 32KB/part
            xkd_bf = persist.tile([128, NKB * D], bf16, tag="xkd")  # 32KB/part
            w_bf = persist.tile([128, ND * D], bf16, tag="w")       # 16KB/part
            rt = persist.tile([128, ND * QW], bf16, tag="rt")       # 16KB/part
            rfx = persist.tile([128, ND * QB], bf16, tag="rfx")     # 8KB/part
            mk = persist.tile([128, 128], bf16, tag="mk")           # 0.25KB/part
            wv_bf = persist.tile([128, ND * D], bf16, tag="wv")     # 16KB/part
            za = persist.tile([128, ND * QB], bf16, tag="za")       # 8KB/part
            zb = persist.tile([128, ND * QB], bf16, tag="zb")       # 8KB/part
            zf = persist.tile([128, ND * QB], bf16, tag="zf")       # 8KB/part
            p_bf = persist.tile([128, NPC], bf16, tag="p")          # 17KB/part
            # f32 rowsum pre-accumulators (DVE), one per softmax group
            pacc_f = persist.tile([128, QB], fp32, tag="paf")       # 2KB/part
            pacc_a = persist.tile([128, QB], fp32, tag="paa")       # 2KB/part
            pacc_b = persist.tile([128, QB], fp32, tag="pab")       # 2KB/part

            # rotating scratch (per-tag bufs)
            scr = root.enter_context(tc.tile_pool(name="scr", bufs=2))
            # one PSUM pool: shared rotating tag (5 banks) + 3 rowsum banks
            psum = root.enter_context(
                tc.tile_pool(name="psum", bufs=5, space="PSUM"))

            for rep in range(body_reps):
                # ---------- phase A: load, Q-proj, R-proj ----------
                # DMA order: tiny first chunks, then wq interleaved with
                # the xt q-columns that Q-proj consumes first. (All loads on
                # the SP queue: the opening is HWDGE-descriptor-generation
                # bound, shared across queues, so striping SP/Act does not
                # raise delivery rate — measured in TimelineSim.)
                xt_v = xt_bf[:].rearrange("p (a s) -> p a s", a=ND)
                nc.sync.dma_start(w_bf[:, 0:128], g_r[0][:, 0:128])
                nc.sync.dma_start(
                    xt_bf[:, QPOS[0]: QPOS[0] + QB],
                    xT_r[0][:, QPOS[0]:QPOS[0] + QB])
                nc.sync.dma_start(w_bf[:, 128:D], g_r[0][:, 128:D])
                for a in range(1, ND):
                    nc.sync.dma_start(w_bf[:, a * D:(a + 1) * D], g_r[a])
                    nc.sync.dma_start(
                        xt_bf[:, a * S + QPOS[0]: a * S + QPOS[0] + QB],
                        xT_r[a][:, QPOS[0]:QPOS[0] + QB])
                for a in range(ND):
                    nc.sync.dma_start(
                        xt_bf[:, a * S + QPOS[1]: a * S + QPOS[1] + QB],
                        xT_r[a][:, QPOS[1]:QPOS[1] + QB])
                nc.sync.dma_start(
                    xt_v[:, :, QB:QPOS[1]], xT_p[:, :, QB:QPOS[1]])
                if rep == 0:
                    nc.sync.dma_start(ws[:], wsel[:])
                    nc.sync.dma_start(wls_a[:], wlsel[0:1, :])
                    nc.sync.dma_start(wls_b[:], wlsel[1:2, :])

                # xkd, wv, mask batched (consumed much later)
                nc.sync.dma_start(
                    xkd_bf[:].rearrange("p (kb d) -> p kb d", kb=NKB),
                    xkd_p)
                nc.sync.dma_start(
                    wv_bf[:].rearrange("p (a d) -> p a d", a=ND), wv_p)
                if rep == 0:
                    nc.sync.dma_start(mk[:], trimask)

                # R [din, q] = G^T @ x_q^T with G = Wk^T Wq folded on host.
                # a-outer over 4-bank din groups: the first matmul needs only
                # the first w/xt chunk, so PE starts ~1 us into the load
                # instead of after the full 2 MB of w.
                for qc in range(2):
                    qp = QPOS[qc]
                    for dg in range(0, ND, 4):
                        pts = [psum.tile([128, QB], fp32, tag="ps",
                                         name=f"pr{qc}_{dg}_{di}")
                               for di in range(4)]
                        for a in range(ND):
                            for di in range(4):
                                nc.tensor.matmul(
                                    pts[di][:],
                                    w_bf[:, a * D + (dg + di) * 128:
                                         a * D + (dg + di) * 128 + 128],
                                    xt_bf[:, a * S + qp: a * S + qp + QB],
                                    start=(a == 0), stop=(a == ND - 1))
                        for di in range(4):
                            din = dg + di
                            nc.vector.tensor_copy(
                                rt[:, din * QW + qc * QB:
                                   din * QW + qc * QB + QB],
                                pts[di][:])

                # rflex = wa*R_A + wb*R_B (wa, wb in {0,1} from data).
                # All on DVE: Act's in-order stream must stay clear for the
                # score exps (PSUM backpressure).
                for a in range(ND):
                    ra = rt[:, a * QW: a * QW + QB]
                    rb = rt[:, a * QW + QB: a * QW + 2 * QB]
                    t1 = scr.tile([128, QB], bf16, tag="t1")
                    nc.vector.tensor_scalar_mul(t1[:], ra, ws[:, 0:1])
                    t2 = scr.tile([128, QB], bf16, tag="t2")
                    nc.vector.tensor_scalar_mul(t2[:], rb, ws[:, 1:2])
                    nc.vector.tensor_add(
                        rfx[:, a * QB:(a + 1) * QB], t1[:], t2[:])

                # ---------- phase B1: scores + exp + rowsums ----------
                def rsrc_ap(qs, a):
                    # qs: 0 = R_A, 1 = R_B, 2 = rflex
                    if qs == 0:
                        return rt[:, a * QW: a * QW + QB]
                    if qs == 1:
                        return rt[:, a * QW + QB: a * QW + 2 * QB]
                    return rfx[:, a * QB:(a + 1) * QB]

                def pfull_ap(fcol):
                    return p_bf[:, fcol * QB:(fcol + 1) * QB]

                def psub_ap(g, qj, i):
                    c = SUBBASE + _sidx(g, qj, i) * 128
                    return p_bf[:, c:c + 128]

                # diagonal sub-tiles at 128 granularity (A group first —
                # R_A lands first), packed 4 per PSUM bank
                def sub_scores(g):
                    grp = [(qj, i) for qj in range(4) for i in range(qj + 1)]
                    for base in range(0, len(grp), 4):
                        pack = grp[base:base + 4]
                        pst = psum.tile([128, QB], fp32, tag="ps")
                        for r, (qj, i) in enumerate(pack):
                            kb = (0 if g == 0 else 12) + i
                            for a in range(ND):
                                rs = rsrc_ap(g, a)
                                nc.tensor.matmul(
                                    pst[:, r * 128:(r + 1) * 128],
                                    xt_bf[:, a * S + kb * 128: a * S + kb * 128 + 128],
                                    rs[:, qj * 128:(qj + 1) * 128],
                                    start=(a == 0), stop=(a == ND - 1),
                                    skip_group_check=True)
                        for r, (qj, i) in enumerate(pack):
                            pc = psub_ap(g, qj, i)
                            nc.scalar.activation(
                                pc, pst[:, r * 128:(r + 1) * 128],
                                mybir.ActivationFunctionType.Exp,
                                scale=_SCALE)
                            if i == qj:
                                nc.vector.tensor_mul(pc, pc, mk[:])

                sub_scores(0)
                for kb, qs, fcol in FSCHED:
                    pst = psum.tile([128, QB], fp32, tag="ps")
                    for a in range(ND):
                        nc.tensor.matmul(
                            pst[:],
                            xt_bf[:, a * S + kb * 128: a * S + kb * 128 + 128],
                            rsrc_ap(qs, a),
                            start=(a == 0), stop=(a == ND - 1))
                    nc.scalar.activation(
                        pfull_ap(fcol), pst[:],
                        mybir.ActivationFunctionType.Exp,
                        scale=_SCALE)
                    if kb == GBF[-1] and qs == 1:
                        sub_scores(1)

                # rowsums: DVE pre-accumulates the P tiles per group (f32,
                # off the PE critical path), then one ones-matmul per group
                # reduces partitions — 3x512 PE cycles instead of 8704.
                def accsubs(pacc, g):
                    for qj in range(4):
                        seg = pacc[:, qj * 128:(qj + 1) * 128]
                        for i in range(qj + 1):
                            if g == 0 and i == 0:
                                nc.vector.tensor_copy(seg, psub_ap(g, qj, i))
                            else:
                                nc.vector.tensor_add(
                                    seg, seg, psub_ap(g, qj, i))

                def lreduce(pacc, tag, btag):
                    pb = scr.tile([128, QB], bf16, tag=btag, bufs=1,
                                  name=f"pb_{btag}")
                    nc.vector.tensor_copy(pb[:], pacc[:])
                    plt = psum.tile([1, QB], fp32, tag=tag, bufs=1,
                                    name=f"l_{tag}")
                    nc.tensor.matmul(plt[:], ones_bf[:], pb[:],
                                     start=True, stop=True)
                    return plt

                # ------- phase B2: Z = x^T P (+combine), O = Wv Z -------
                def xk_ap(kb, dsl):
                    return xkd_bf[:, kb * D + dsl * 128: kb * D + dsl * 128 + 128]

                def zsubs(pz, g, dsl, start_open):
                    for qj in range(4):
                        for i in range(qj + 1):
                            kb = (0 if g == 0 else 12) + i
                            nc.tensor.matmul(
                                pz[:, qj * 128:(qj + 1) * 128],
                                xk_ap(kb, dsl), psub_ap(g, qj, i),
                                start=(i == 0 and not start_open),
                                stop=(i == qj),
                                skip_group_check=True)

                def zcombine(zt, pz, dsl, wcol):
                    tf = scr.tile([128, QB], bf16, tag="tf")
                    nc.vector.tensor_scalar_mul(
                        tf[:], zf[:, dsl * QB:(dsl + 1) * QB],
                        ws[:, wcol:wcol + 1])
                    nc.vector.tensor_add(
                        zt[:, dsl * QB:(dsl + 1) * QB], pz[:], tf[:])

                # Z_F first (plain copy; zf feeds both combines)
                for dsl in range(ND):
                    pz = psum.tile([128, QB], fp32, tag="ps")
                    for j, kb in enumerate(GF):
                        nc.tensor.matmul(
                            pz[:], xk_ap(kb, dsl), pfull_ap(j),
                            start=(j == 0), stop=(j == 3))
                    nc.vector.tensor_copy(
                        zf[:, dsl * QB:(dsl + 1) * QB], pz[:])

                # Z_A: sub-tiles only
                for dsl in range(ND):
                    pz = psum.tile([128, QB], fp32, tag="ps")
                    zsubs(pz, 0, dsl, start_open=False)
                    zcombine(za, pz, dsl, 0)

                # rowsum pre-accumulation on DVE — emitted here so it drains
                # during Z_B's long PE chains instead of stalling Z_A
                accsubs(pacc_a, 0)
                nc.vector.tensor_add(
                    pacc_b[:], pfull_ap(4), pfull_ap(5))
                for j in range(2, len(GBF)):
                    nc.vector.tensor_add(
                        pacc_b[:], pacc_b[:], pfull_ap(4 + j))
                accsubs(pacc_b, 1)
                nc.vector.tensor_add(pacc_f[:], pfull_ap(0), pfull_ap(1))
                for j in range(2, 4):
                    nc.vector.tensor_add(
                        pacc_f[:], pacc_f[:], pfull_ap(j))

                # Z_B: full part then sub-tiles
                for dsl in range(ND):
                    pz = psum.tile([128, QB], fp32, tag="ps")
                    for j, kb in enumerate(GBF):
                        nc.tensor.matmul(
                            pz[:], xk_ap(kb, dsl), pfull_ap(4 + j),
                            start=(j == 0), stop=False,
                            skip_group_check=True)
                    zsubs(pz, 1, dsl, start_open=True)
                    zcombine(zb, pz, dsl, 1)

                # rowsum reduce + lsum store (PE cost: 3x512 cycles)
                l_f = lreduce(pacc_f, "lf", "lfb")
                l_a = lreduce(pacc_a, "la", "lab")
                l_b = lreduce(pacc_b, "lb", "lbb")
                for s, l_s, wl in ((0, l_a, wls_a), (1, l_b, wls_b)):
                    tl = scr.tile([1, QB], fp32, tag="tl")
                    nc.vector.tensor_mul(tl[:], l_f[:], wl[:])
                    lo = scr.tile([1, QB], fp32, tag="lo")
                    nc.vector.tensor_add(lo[:], l_s[:], tl[:])
                    # stores go on the Activation HWDGE queue so the SP
                    # queue's next-rep input loads aren't head-blocked
                    nc.scalar.dma_start(lsum[s:s + 1, :], lo[:])

                # O^T [dv, q] = Wv @ Z (unnormalized; host divides).
                # dv-outer: both q-slots of a dv pair into ONE [128,1024]
                # tile and one store — 8 stores instead of 16 halves the
                # HWDGE store-stream processing that paces the O phase at
                # real (faster-than-model) PE clocks; 3-deep rotation keeps
                # chain pairs from stalling on the 512KB stores.
                for dv in range(ND):
                    ot = scr.tile([128, 2 * QB], fp32, tag="ot", bufs=3)
                    for s, zt in ((0, za), (1, zb)):
                        po = psum.tile([128, QB], fp32, tag="ps")
                        for a in range(ND):
                            nc.tensor.matmul(
                                po[:],
                                wv_bf[:, a * D + dv * 128: a * D + dv * 128 + 128],
                                zt[:, a * QB:(a + 1) * QB],
                                start=(a == 0), stop=(a == ND - 1))
                        nc.vector.tensor_copy(
                            ot[:, s * QB:(s + 1) * QB], po[:])
                    nc.scalar.dma_start(outT_r[dv], ot[:])

    nc.compile()
    return nc


_NC_CACHE = {}


def _get_nc(body_reps=1):
    if body_reps not in _NC_CACHE:
        _NC_CACHE[body_reps] = build_nc(body_reps)
    return _NC_CACHE[body_reps]


def make_in_maps(x, Wq, Wk, Wv):
    """Host-side sharding: per-core input dict."""
    x = np.asarray(x, dtype=np.float32)
    gTn = np.ascontiguousarray(
        np.asarray(Wq, np.float64).T @ np.asarray(Wk, np.float64)
    ).astype(ml_dtypes.bfloat16)
    wvT = np.ascontiguousarray(np.asarray(Wv, np.float32).T).astype(ml_dtypes.bfloat16)

    # shared lower-tri (keep k <= q) mask for every diagonal 128x128 block
    tri = (np.arange(128)[:, None] <= np.arange(128)[None, :]).astype(
        ml_dtypes.bfloat16)

    in_maps = []
    for core in range(N_CORES):
        b, qbs, gather, wa = _core_layout(core)
        xg = x[b][gather]                                    # [S, D] gathered
        xkd = np.ascontiguousarray(xg).astype(ml_dtypes.bfloat16)
        xTp = np.ascontiguousarray(xg.T).astype(ml_dtypes.bfloat16)
        wsel = np.zeros((KB, 2), np.float32)
        wsel[:, 0] = wa
        wsel[:, 1] = 1.0 - wa
        wlsel = np.zeros((2, QB), np.float32)
        wlsel[0, :] = wa
        wlsel[1, :] = 1.0 - wa
        in_maps.append({
            "xT": xTp,
            "xkd": xkd,
            "gT": gTn,
            "wvT": wvT,
            "trimask": tri,
            "wsel": wsel,
            "wlsel": wlsel,
        })
    return in_maps


def assemble_output(results):
    out = np.zeros((B, S, D), np.float32)
    for core in range(N_CORES):
        b, qbs, _, _ = _core_layout(core)
        outT = results[core]["outT"]      # [D, 1024] unnormalized
        l = results[core]["lsum"]         # [2, QB]
        for slot in range(2):
            rows = np.arange(qbs[slot] * QB, (qbs[slot] + 1) * QB)
            o = outT[:, slot * QB:(slot + 1) * QB].T   # [QB, D]
            out[b, rows, :] = o / l[slot][:, None]
    return out


def kernel(x, Wq, Wk, Wv):
    from concourse.bass_utils import run_bass_kernel_spmd
    nc = _get_nc()
    in_maps = make_in_maps(x, Wq, Wk, Wv)
    res = run_bass_kernel_spmd(nc, in_maps, core_ids=list(range(N_CORES)))
    return assemble_output(res.results)

